# revision 1
# baseline (speedup 1.0000x reference)
"""CondConv2d (MoE-routed 3x3 conv) Trainium2 Bass kernel.

Full-input contract: kernel(**inputs) takes the unsharded tensors and
returns the full [32, 192, 56, 56] output. Internally: data-parallel
across batch over 8 NeuronCores (4 samples per core). Each core mixes
its own per-sample weights on-device (routing @ expert_weights via
chained DVE multiply-accumulate; experts replicated in SBUF) and runs
its samples' convolutions as shifted float32r matmuls accumulating in
PSUM; PSUM is drained by the scalar engine with the routed bias fused
into the copy, keeping the vector engine free for mixing.

Conv decomposition per sample (I=O=192, K=3, H=W=56, pad=1):
  out[o, p] = sum over (i, dy, dx) of w[o, i, dy, dx] * xpad[i, h+dy, w+dx]
As matmuls with contraction on the SBUF partition dim (<=128):
  - channels i in [0,128): 9 chunks (one per (dy,dx)), K=128
  - channels i in [128,192): stored twice in one tile, second copy
    pre-shifted one row, so a single K=128 matmul covers (dy=0, dy=1)
    for a given dx -> 3 paired chunks; dy=2 is 3 more K=64 chunks.
  => 15 accumulating matmuls per (O-chunk, pixel-tile); O split 128+64.
Pixels tiled 7 x 448 (8 rows of 56), each tile in its own PSUM bank.

Weights arrive offset-group-major so sample 0's first head chunks can
start as soon as the first group's experts are mixed; mixing and x DMA
for sample b+1 are emitted ahead of sample b's conv so the DVE stream
runs a full sample ahead of the PE.
"""

import numpy as np

B, E = 32, 8
O, I = 192, 192
H, W = 56, 56
HP = H + 2  # padded side
NCORES = 8
BPC = B // NCORES  # samples per core
NT = 7  # pixel tiles per sample
RPT = 8  # output rows per pixel tile
TW = RPT * W  # 448 pixels per tile
NCH = 15  # accumulating matmul chunks per (O-chunk, pixel-tile)
OC = ((0, 128), (128, 64))  # (o_start, o_size) chunks

_CACHE = {}


def _build():
    import concourse.bass as bass  # noqa: F401
    from concourse import bacc, mybir, tile

    dt = mybir.dt
    f32 = dt.float32
    f32r = dt.float32r
    MULT = mybir.AluOpType.mult
    ADD = mybir.AluOpType.add
    IDENT = mybir.ActivationFunctionType.Identity

    nc = bacc.Bacc(
        "TRN2",
        target_bir_lowering=False,
        debug=False,
        enable_asserts=False,
        num_devices=NCORES,
    )

    xin = nc.dram_tensor("xin", [BPC, I, H, W], f32, kind="ExternalInput").ap()
    # wht free layout: ((g*E + e)*3 + d)*O + o with off = g*3 + d
    wht_d = nc.dram_tensor("wht", [128, 9 * E * O], f32, kind="ExternalInput").ap()
    # wtp/wt2 free layout: e*(3*O) + dx*O + o
    wtp_d = nc.dram_tensor("wtp", [128, 3 * E * O], f32, kind="ExternalInput").ap()
    wt2_d = nc.dram_tensor("wt2", [64, 3 * E * O], f32, kind="ExternalInput").ap()
    bias_d = nc.dram_tensor("bias", [E, O], f32, kind="ExternalInput").ap()
    rt_d = nc.dram_tensor("rt", [E, BPC], f32, kind="ExternalInput").ap()
    rf_d = nc.dram_tensor("rf", [1, BPC * E], f32, kind="ExternalInput").ap()
    out_d = nc.dram_tensor("out", [BPC, O, H * W], f32, kind="ExternalOutput").ap()

    with tile.TileContext(nc) as tc:
        with (
            tc.tile_pool(name="consts", bufs=1) as consts,
            tc.tile_pool(name="wm", bufs=3) as wm_pool,
            tc.tile_pool(name="stage", bufs=4) as stage_pool,
            tc.tile_pool(name="psum1", bufs=1, space="PSUM") as psum1,
            tc.tile_pool(name="cpsum", bufs=1, space="PSUM") as cpsum,
        ):
            # ---- small inputs (needed for broadcast/bias matmuls)
            bias_sb = consts.tile([E, O], f32)
            nc.sync.dma_start(out=bias_sb, in_=bias_d)
            rt_sb = consts.tile([E, BPC], f32)
            nc.sync.dma_start(out=rt_sb, in_=rt_d)
            rf_sb = consts.tile([1, BPC * E], f32)
            nc.sync.dma_start(out=rf_sb, in_=rf_d)

            # ---- broadcast routing to all partitions + mix bias, via matmuls
            ones_sb = consts.tile([1, 128], f32)
            nc.vector.memset(ones_sb, 1.0)
            NRB = BPC * E
            ps0 = psum1.tile([128, 2 * BPC + NRB], f32)  # one PSUM bank
            nc.tensor.matmul(ps0[:, 0:BPC], lhsT=bias_sb[:, 0:128], rhs=rt_sb,
                             start=True, stop=True)
            nc.tensor.matmul(ps0[0:64, BPC:2 * BPC], lhsT=bias_sb[:, 128:192],
                             rhs=rt_sb, start=True, stop=True)
            nc.tensor.matmul(ps0[:, 2 * BPC:], lhsT=ones_sb, rhs=rf_sb,
                             start=True, stop=True)
            bias_cols = consts.tile([128, 2 * BPC], f32)
            nc.vector.tensor_copy(bias_cols[:, 0:BPC], ps0[:, 0:BPC])
            nc.vector.tensor_copy(bias_cols[0:64, BPC:2 * BPC], ps0[0:64, BPC:2 * BPC])
            rb = consts.tile([128, NRB], f32)
            nc.vector.tensor_copy(rb, ps0[:, 2 * BPC:])

            # ---- persistent padded-x tiles first: sample 0's x must not
            # queue behind 10.6MB of weight DMA.
            xh = [consts.tile([128, HP, HP], f32r, tag=f"xh{i}", name=f"xh{i}")
                  for i in range(2)]
            xt = [consts.tile([128, HP, HP], f32r, tag=f"xt{i}", name=f"xt{i}")
                  for i in range(2)]
            for t_ in xh + xt:
                nc.vector.memset(t_[:, 0, :].bitcast(f32), 0.0)
                nc.vector.memset(t_[:, HP - 1, :].bitcast(f32), 0.0)
                nc.vector.memset(t_[:, :, 0].bitcast(f32), 0.0)
                nc.vector.memset(t_[:, :, HP - 1].bitcast(f32), 0.0)

            def emit_x_dma(b):
                xhb, xtb = xh[b % 2], xt[b % 2]
                nc.sync.dma_start(out=xhb[:, 1:H + 1, 1:W + 1],
                                  in_=xin[b, 0:128].bitcast(f32r))
                nc.sync.dma_start(out=xtb[0:64, 1:H + 1, 1:W + 1],
                                  in_=xin[b, 128:192].bitcast(f32r))
                nc.sync.dma_start(out=xtb[64:128, 0:H, 1:W + 1],
                                  in_=xin[b, 128:192].bitcast(f32r))

            emit_x_dma(0)

            # ---- resident expert weights, offset-group-major arrival order
            wht = consts.tile([128, 3, 3 * E * O], f32)  # [i, g, (e, d, o)]
            wtp = consts.tile([128, E, 3 * O], f32)  # [i2pair, e, (dx, o)]
            wt2 = consts.tile([64, E, 3 * O], f32)
            for g in range(3):
                for e in range(E):
                    base = (g * E + e) * (3 * O)
                    nc.sync.dma_start(out=wht[:, g, e * 3 * O:(e + 1) * 3 * O],
                                      in_=wht_d[:, base:base + 3 * O])
            for e in range(E):
                nc.sync.dma_start(out=wtp[:, e, :],
                                  in_=wtp_d[:, e * 3 * O:(e + 1) * 3 * O])
                nc.sync.dma_start(out=wt2[:, e, :],
                                  in_=wt2_d[:, e * 3 * O:(e + 1) * 3 * O])

            def emit_mix_head(b):
                # wmh free = off*O + o
                wmh = wm_pool.tile([128, 9 * O], f32r, tag="wmh", name="wmh")
                for g in range(3):  # independent chains per offset group
                    dst = wmh[:, g * 3 * O:(g + 1) * 3 * O]
                    for e in range(E):
                        rc = rb[:, b * E + e:b * E + e + 1]
                        srcw = wht[:, g, e * 3 * O:(e + 1) * 3 * O]
                        if e == 0:
                            nc.vector.tensor_scalar_mul(dst, srcw, rc)
                        else:
                            nc.vector.scalar_tensor_tensor(
                                dst, srcw, rc, dst, op0=MULT, op1=ADD)
                return wmh

            def emit_mix_tail(b):
                # wmp/wm2 free = dx*O + o
                wmp = wm_pool.tile([128, 3 * O], f32r, tag="wmp", name="wmp")
                wm2 = wm_pool.tile([64, 3 * O], f32r, tag="wm2", name="wm2")
                for e in range(E):
                    rc = rb[:, b * E + e:b * E + e + 1]
                    rc64 = rb[0:64, b * E + e:b * E + e + 1]
                    if e == 0:
                        nc.vector.tensor_scalar_mul(wmp, wtp[:, 0, :], rc)
                        nc.vector.tensor_scalar_mul(wm2, wt2[:, 0, :], rc64)
                    else:
                        nc.vector.scalar_tensor_tensor(
                            wmp, wtp[:, e, :], rc, wmp, op0=MULT, op1=ADD)
                        nc.vector.scalar_tensor_tensor(
                            wm2, wt2[:, e, :], rc64, wm2, op0=MULT, op1=ADD)
                return wmp, wm2

            def emit_mix(b):
                return (emit_mix_head(b), *emit_mix_tail(b))

            wm = {0: emit_mix(0)}

            for b in range(BPC):
                if b + 1 < BPC:
                    emit_x_dma(b + 1)
                    wm[b + 1] = emit_mix(b + 1)
                xhb, xtb = xh[b % 2], xt[b % 2]
                wmh, wmp, wm2 = wm.pop(b)

                for oci, (o0, osz) in enumerate(OC):
                    pst = [cpsum.tile([128, 512], f32, tag=f"cps{t}",
                                      name=f"cps{t}") for t in range(NT)]
                    ci = 0
                    # head: channels 0:128, one chunk per (dy, dx), K=128
                    for dy in range(3):
                        for dx in range(3):
                            off = dy * 3 + dx
                            lhsT = wmh[:, off * O + o0:off * O + o0 + osz]
                            for t in range(NT):
                                nc.tensor.matmul(
                                    pst[t][0:osz, 0:TW],
                                    lhsT=lhsT,
                                    rhs=xhb[:, t * RPT + dy:t * RPT + dy + RPT,
                                            dx:dx + W],
                                    start=(ci == 0), stop=(ci == NCH - 1))
                            ci += 1
                    # tail paired: channels 128:192, (dy=0,1) pairs, K=128
                    for dx in range(3):
                        lhsT = wmp[:, dx * O + o0:dx * O + o0 + osz]
                        for t in range(NT):
                            nc.tensor.matmul(
                                pst[t][0:osz, 0:TW],
                                lhsT=lhsT,
                                rhs=xtb[:, t * RPT:t * RPT + RPT, dx:dx + W],
                                start=(ci == 0), stop=(ci == NCH - 1))
                        ci += 1
                    # tail dy=2: channels 128:192, K=64
                    for dx in range(3):
                        lhsT = wm2[0:64, dx * O + o0:dx * O + o0 + osz]
                        for t in range(NT):
                            nc.tensor.matmul(
                                pst[t][0:osz, 0:TW],
                                lhsT=lhsT,
                                rhs=xtb[0:64, t * RPT + 2:t * RPT + 2 + RPT,
                                        dx:dx + W],
                                start=(ci == 0), stop=(ci == NCH - 1))
                        ci += 1
                    assert ci == NCH

                    # ---- drain PSUM -> SBUF on ScalarE with fused bias
                    bc = (bias_cols[:, b:b + 1] if osz == 128
                          else bias_cols[0:64, BPC + b:BPC + b + 1])
                    for t in range(NT):
                        st = stage_pool.tile([128, TW], f32, tag="st", name="st")
                        nc.scalar.activation(
                            st[0:osz], pst[t][0:osz, 0:TW], IDENT, bias=bc)
                        nc.sync.dma_start(
                            out=out_d[b, o0:o0 + osz, t * TW:(t + 1) * TW],
                            in_=st[0:osz])

    nc.compile()
    return nc


def _prep_inputs(x, routing_weights, weight, bias):
    x = np.asarray(x, np.float32)
    routing = np.asarray(routing_weights, np.float32)
    weight = np.asarray(weight, np.float32)
    bias = np.asarray(bias, np.float32)

    W5 = weight.reshape(E, O, I, 3, 3)
    # head: [i, dy(g), e, dx(d), o] -> f = ((g*E + e)*3 + d)*O + o
    wht_h = np.ascontiguousarray(
        W5[:, :, :128].transpose(2, 3, 0, 4, 1)).reshape(128, 9 * E * O)
    # tail pair: p<64 -> (i=128+p, dy=0); p>=64 -> (i=64+p, dy=1)
    # f = e*(3*O) + dx*O + o
    t0 = W5[:, :, 128:, 0, :].transpose(2, 0, 3, 1)  # [i2, e, dx, o]
    t1 = W5[:, :, 128:, 1, :].transpose(2, 0, 3, 1)
    wtp_h = np.ascontiguousarray(
        np.concatenate([t0, t1], axis=0)).reshape(128, 3 * E * O)
    wt2_h = np.ascontiguousarray(
        W5[:, :, 128:, 2, :].transpose(2, 0, 3, 1)).reshape(64, 3 * E * O)

    in_maps = []
    for c in range(NCORES):
        sl = slice(c * BPC, (c + 1) * BPC)
        in_maps.append({
            "xin": np.ascontiguousarray(x[sl]),
            "wht": wht_h,
            "wtp": wtp_h,
            "wt2": wt2_h,
            "bias": bias,
            "rt": np.ascontiguousarray(routing[sl].T),
            "rf": np.ascontiguousarray(routing[sl].reshape(1, BPC * E)),
        })
    return in_maps


def _run(in_maps, **kwargs):
    from concourse import bass_utils
    if "nc" not in _CACHE:
        _CACHE["nc"] = _build()
    return bass_utils.run_bass_kernel_spmd(
        _CACHE["nc"], in_maps, core_ids=list(range(NCORES)), **kwargs)


def kernel(x, routing_weights, weight, bias):
    in_maps = _prep_inputs(x, routing_weights, weight, bias)
    res = _run(in_maps)
    out = np.stack([res.results[c]["out"] for c in range(NCORES)], axis=0)
    return out.reshape(B, O, H, W)



# revision 2
# speedup vs baseline: 1.0487x; 1.0487x over previous
"""CondConv2d (MoE-routed 3x3 conv) Trainium2 Bass kernel.

Full-input contract: kernel(**inputs) takes the unsharded tensors and
returns the full [32, 192, 56, 56] output. Internally: data-parallel
across batch over 8 NeuronCores (4 samples per core). Each core mixes
its own per-sample weights on-device (routing @ expert_weights via
chained DVE multiply-accumulate; experts replicated in SBUF) and runs
its samples' convolutions as shifted bf16 matmuls accumulating in
fp32 PSUM; PSUM is drained by the scalar engine with the routed bias
fused into the copy.

Conv decomposition per sample (I=O=192, K=3, H=W=56, pad=1):
  out[o, p] = sum over (i, dy, dx) of w[o, i, dy, dx] * xpad[i, h+dy, w+dx]
As matmuls with contraction on the SBUF partition dim (<=128):
  - channels i in [0,128): 9 chunks (one per (dy,dx)), K=128
  - channels i in [128,192) stored as three shifted 2-copy tiles so
    pairs of offsets share one K=128 matmul:
      xt = [normal | row-shifted]  -> (dy=0,dy=1) pairs per dx: 3 chunks
      xc = [normal | col-shifted]  -> (dy=2,dx=0)|(dy=2,dx=1): 1 chunk
      leftover (dy=2,dx=2): 1 chunk, K=64 (reads xt[0:64])
  => 14 accumulating matmuls per (O-chunk, pixel-tile); O split 128+64.
Pixels tiled 7 x 448 (8 rows of 56), each tile in its own PSUM bank.

Everything on the matmul path is bf16 (x, expert weights, mixed
weights, staged output); routing scalars, bias and PSUM stay fp32.
Weights arrive offset-group-major and ahead of x-tails so sample 0's
first chunks start ~8us in; mixing and x DMA for sample b+1 are
emitted ahead of sample b's conv so the DVE stream runs a full sample
ahead of the PE. Output is staged per (sample, O-chunk) and written
with one large DMA to keep HWDGE acquisitions low.
"""

import numpy as np

B, E = 32, 8
O, I = 192, 192
H, W = 56, 56
HP = H + 2  # padded side
NCORES = 8
BPC = B // NCORES  # samples per core
NT = 7  # pixel tiles per sample
RPT = 8  # output rows per pixel tile
TW = RPT * W  # 448 pixels per tile
NCH = 14  # accumulating matmul chunks per (O-chunk, pixel-tile)
OC = ((0, 128), (128, 64))  # (o_start, o_size) chunks

_CACHE = {}


def _build():
    import concourse.bass as bass  # noqa: F401
    from concourse import bacc, mybir, tile

    dt = mybir.dt
    f32 = dt.float32
    bf16 = dt.bfloat16
    MULT = mybir.AluOpType.mult
    ADD = mybir.AluOpType.add
    IDENT = mybir.ActivationFunctionType.Identity

    nc = bacc.Bacc(
        "TRN2",
        target_bir_lowering=False,
        debug=False,
        enable_asserts=False,
        num_devices=NCORES,
    )

    xin = nc.dram_tensor("xin", [BPC, I, H, W], bf16, kind="ExternalInput").ap()
    # wht free layout: ((g*E + e)*3 + d)*O + o with off = g*3 + d
    wht_d = nc.dram_tensor("wht", [128, 9 * E * O], bf16, kind="ExternalInput").ap()
    # wtp free layout: e*(3*O) + dx*O + o; p<64: (ch 128+p, dy0); p>=64: dy1
    wtp_d = nc.dram_tensor("wtp", [128, 3 * E * O], bf16, kind="ExternalInput").ap()
    # wcp free layout: e*O + o; p<64: (ch 128+p, dy2, dx0); p>=64: (dy2, dx1)
    wcp_d = nc.dram_tensor("wcp", [128, E * O], bf16, kind="ExternalInput").ap()
    # ws2 free layout: e*O + o; (ch 128+p, dy2, dx2)
    ws2_d = nc.dram_tensor("ws2", [64, E * O], bf16, kind="ExternalInput").ap()
    bias_d = nc.dram_tensor("bias", [E, O], f32, kind="ExternalInput").ap()
    rt_d = nc.dram_tensor("rt", [E, BPC], f32, kind="ExternalInput").ap()
    rf_d = nc.dram_tensor("rf", [1, BPC * E], f32, kind="ExternalInput").ap()
    out_d = nc.dram_tensor("out", [BPC, O, H * W], bf16, kind="ExternalOutput").ap()

    with tile.TileContext(nc) as tc:
        with (
            tc.tile_pool(name="consts", bufs=1) as consts,
            tc.tile_pool(name="wm", bufs=3) as wm_pool,
            tc.tile_pool(name="stage", bufs=2) as stage_pool,
            tc.tile_pool(name="psum1", bufs=1, space="PSUM") as psum1,
            tc.tile_pool(name="cpsum", bufs=1, space="PSUM") as cpsum,
        ):
            # ---- small inputs (needed for broadcast/bias matmuls)
            bias_sb = consts.tile([E, O], f32)
            nc.sync.dma_start(out=bias_sb, in_=bias_d)
            rt_sb = consts.tile([E, BPC], f32)
            nc.sync.dma_start(out=rt_sb, in_=rt_d)
            rf_sb = consts.tile([1, BPC * E], f32)
            nc.sync.dma_start(out=rf_sb, in_=rf_d)

            # ---- broadcast routing to all partitions + mix bias, via matmuls
            ones_sb = consts.tile([1, 128], f32)
            nc.vector.memset(ones_sb, 1.0)
            NRB = BPC * E
            ps0 = psum1.tile([128, 2 * BPC + NRB], f32)  # one PSUM bank
            nc.tensor.matmul(ps0[:, 0:BPC], lhsT=bias_sb[:, 0:128], rhs=rt_sb,
                             start=True, stop=True)
            nc.tensor.matmul(ps0[0:64, BPC:2 * BPC], lhsT=bias_sb[:, 128:192],
                             rhs=rt_sb, start=True, stop=True)
            nc.tensor.matmul(ps0[:, 2 * BPC:], lhsT=ones_sb, rhs=rf_sb,
                             start=True, stop=True)
            bias_cols = consts.tile([128, 2 * BPC], f32)
            nc.vector.tensor_copy(bias_cols[:, 0:BPC], ps0[:, 0:BPC])
            nc.vector.tensor_copy(bias_cols[0:64, BPC:2 * BPC], ps0[0:64, BPC:2 * BPC])
            rb = consts.tile([128, NRB], f32)
            nc.vector.tensor_copy(rb, ps0[:, 2 * BPC:])

            # ---- resident expert weights + per-sample x tiles.
            # DMA queue order is the critical path at startup:
            #   wht g0 -> xh0 -> wht g1, g2 -> xt0, xc0 -> wtp, wcp, ws2
            wht = consts.tile([128, 3, 3 * E * O], bf16)  # [i, g, (e, d, o)]
            wtp = consts.tile([128, E, 3 * O], bf16)  # [i2pair, e, (dx, o)]
            wcp = consts.tile([128, E, O], bf16)  # [i2cpair, e, o]
            ws2 = consts.tile([64, E, O], bf16)

            def emit_wht_dma(g):
                for e in range(E):
                    base = (g * E + e) * (3 * O)
                    nc.sync.dma_start(out=wht[:, g, e * 3 * O:(e + 1) * 3 * O],
                                      in_=wht_d[:, base:base + 3 * O])

            xh = [consts.tile([128, HP, HP], bf16, tag=f"xh{i}", name=f"xh{i}")
                  for i in range(2)]
            xt = [consts.tile([128, HP, HP], bf16, tag=f"xt{i}", name=f"xt{i}")
                  for i in range(2)]
            xc = [consts.tile([128, HP, HP], bf16, tag=f"xc{i}", name=f"xc{i}")
                  for i in range(2)]
            for t_ in xh + xt + xc:
                nc.vector.memset(t_[:, 0, :], 0.0)
                nc.vector.memset(t_[:, HP - 1, :], 0.0)
                nc.vector.memset(t_[:, :, 0], 0.0)
                nc.vector.memset(t_[:, :, HP - 1], 0.0)

            def emit_xh_dma(b):
                nc.sync.dma_start(out=xh[b % 2][:, 1:H + 1, 1:W + 1],
                                  in_=xin[b, 0:128])

            def emit_xtail_dma(b):
                xtb, xcb = xt[b % 2], xc[b % 2]
                nc.sync.dma_start(out=xtb[0:64, 1:H + 1, 1:W + 1],
                                  in_=xin[b, 128:192])
                nc.sync.dma_start(out=xtb[64:128, 0:H, 1:W + 1],
                                  in_=xin[b, 128:192])
                nc.sync.dma_start(out=xcb[0:64, 1:H + 1, 1:W + 1],
                                  in_=xin[b, 128:192])
                nc.sync.dma_start(out=xcb[64:128, 1:H + 1, 0:W],
                                  in_=xin[b, 128:192])

            emit_wht_dma(0)
            emit_xh_dma(0)
            emit_wht_dma(1)
            emit_wht_dma(2)
            emit_xtail_dma(0)
            for e in range(E):
                nc.sync.dma_start(out=wtp[:, e, :],
                                  in_=wtp_d[:, e * 3 * O:(e + 1) * 3 * O])
            nc.sync.dma_start(out=wcp, in_=wcp_d)
            nc.sync.dma_start(out=ws2, in_=ws2_d)

            def _mix_chain(dst, srcs, b):
                for e in range(E):
                    rc = rb[0:dst.shape[0], b * E + e:b * E + e + 1]
                    if e == 0:
                        nc.vector.tensor_scalar_mul(dst, srcs(e), rc)
                    else:
                        nc.vector.scalar_tensor_tensor(
                            dst, srcs(e), rc, dst, op0=MULT, op1=ADD)

            def emit_mix(b):
                # per-offset-group head tiles so PE can start after group 0
                wmg = []
                for g in range(3):
                    t_ = wm_pool.tile([128, 3 * O], bf16, tag=f"wmg{g}",
                                      name=f"wmg{g}")
                    _mix_chain(t_, lambda e, g=g: wht[:, g, e * 3 * O:(e + 1) * 3 * O], b)
                    wmg.append(t_)
                wmp = wm_pool.tile([128, 3 * O], bf16, tag="wmp", name="wmp")
                _mix_chain(wmp, lambda e: wtp[:, e, :], b)
                wmc = wm_pool.tile([128, O], bf16, tag="wmc", name="wmc")
                _mix_chain(wmc, lambda e: wcp[:, e, :], b)
                wms = wm_pool.tile([64, O], bf16, tag="wms", name="wms")
                _mix_chain(wms, lambda e: ws2[:, e, :], b)
                return (*wmg, wmp, wmc, wms)

            wm = {0: emit_mix(0)}

            for b in range(BPC):
                if b + 1 < BPC:
                    emit_xh_dma(b + 1)
                    emit_xtail_dma(b + 1)
                    wm[b + 1] = emit_mix(b + 1)
                xhb, xtb, xcb = xh[b % 2], xt[b % 2], xc[b % 2]
                wmg0, wmg1, wmg2, wmp, wmc, wms = wm.pop(b)
                wmg = (wmg0, wmg1, wmg2)

                for oci, (o0, osz) in enumerate(OC):
                    pst = [cpsum.tile([128, 512], f32, tag=f"cps{t}",
                                      name=f"cps{t}") for t in range(NT)]
                    ci = 0
                    # head: channels 0:128, one chunk per (dy, dx), K=128
                    for dy in range(3):
                        for dx in range(3):
                            lhsT = wmg[dy][:, dx * O + o0:dx * O + o0 + osz]
                            for t in range(NT):
                                nc.tensor.matmul(
                                    pst[t][0:osz, 0:TW],
                                    lhsT=lhsT,
                                    rhs=xhb[:, t * RPT + dy:t * RPT + dy + RPT,
                                            dx:dx + W],
                                    start=(ci == 0), stop=(ci == NCH - 1))
                            ci += 1
                    # tail row-pairs: channels 128:192, (dy=0,1) pairs, K=128
                    for dx in range(3):
                        lhsT = wmp[:, dx * O + o0:dx * O + o0 + osz]
                        for t in range(NT):
                            nc.tensor.matmul(
                                pst[t][0:osz, 0:TW],
                                lhsT=lhsT,
                                rhs=xtb[:, t * RPT:t * RPT + RPT, dx:dx + W],
                                start=(ci == 0), stop=(ci == NCH - 1))
                        ci += 1
                    # tail col-pair: (dy=2,dx=0)|(dy=2,dx=1), K=128
                    lhsT = wmc[:, o0:o0 + osz]
                    for t in range(NT):
                        nc.tensor.matmul(
                            pst[t][0:osz, 0:TW],
                            lhsT=lhsT,
                            rhs=xcb[:, t * RPT + 2:t * RPT + 2 + RPT, 0:W],
                            start=(ci == 0), stop=(ci == NCH - 1))
                    ci += 1
                    # tail single: (dy=2, dx=2), K=64
                    lhsT = wms[0:64, o0:o0 + osz]
                    for t in range(NT):
                        nc.tensor.matmul(
                            pst[t][0:osz, 0:TW],
                            lhsT=lhsT,
                            rhs=xtb[0:64, t * RPT + 2:t * RPT + 2 + RPT,
                                    2:2 + W],
                            start=(ci == 0), stop=(ci == NCH - 1))
                    ci += 1
                    assert ci == NCH

                    # ---- drain PSUM -> SBUF on ScalarE with fused bias,
                    # then one batched DMA per (sample, O-chunk)
                    bc = (bias_cols[:, b:b + 1] if osz == 128
                          else bias_cols[0:64, BPC + b:BPC + b + 1])
                    st = stage_pool.tile([128, H * W], bf16, tag="st", name="st")
                    for t in range(NT):
                        nc.scalar.activation(
                            st[0:osz, t * TW:(t + 1) * TW],
                            pst[t][0:osz, 0:TW], IDENT, bias=bc)
                    nc.sync.dma_start(out=out_d[b, o0:o0 + osz, :],
                                      in_=st[0:osz, :])

    nc.compile()
    return nc


def _prep_inputs(x, routing_weights, weight, bias):
    import ml_dtypes
    bf16 = ml_dtypes.bfloat16

    x = np.asarray(x, np.float32).astype(bf16)
    routing = np.asarray(routing_weights, np.float32)
    weight = np.asarray(weight, np.float32).astype(bf16)
    bias = np.asarray(bias, np.float32)

    W5 = weight.reshape(E, O, I, 3, 3)
    # head: [i, dy(g), e, dx(d), o] -> f = ((g*E + e)*3 + d)*O + o
    wht_h = np.ascontiguousarray(
        W5[:, :, :128].transpose(2, 3, 0, 4, 1)).reshape(128, 9 * E * O)
    # tail row pair: p<64 -> (i=128+p, dy=0); p>=64 -> (i=64+p, dy=1)
    # f = e*(3*O) + dx*O + o
    t0 = W5[:, :, 128:, 0, :].transpose(2, 0, 3, 1)  # [i2, e, dx, o]
    t1 = W5[:, :, 128:, 1, :].transpose(2, 0, 3, 1)
    wtp_h = np.ascontiguousarray(
        np.concatenate([t0, t1], axis=0)).reshape(128, 3 * E * O)
    # tail col pair: p<64 -> (dy=2, dx=0); p>=64 -> (dy=2, dx=1); f = e*O + o
    c0 = W5[:, :, 128:, 2, 0].transpose(2, 0, 1)  # [i2, e, o]
    c1 = W5[:, :, 128:, 2, 1].transpose(2, 0, 1)
    wcp_h = np.ascontiguousarray(
        np.concatenate([c0, c1], axis=0)).reshape(128, E * O)
    ws2_h = np.ascontiguousarray(
        W5[:, :, 128:, 2, 2].transpose(2, 0, 1)).reshape(64, E * O)

    in_maps = []
    for c in range(NCORES):
        sl = slice(c * BPC, (c + 1) * BPC)
        in_maps.append({
            "xin": np.ascontiguousarray(x[sl]),
            "wht": wht_h,
            "wtp": wtp_h,
            "wcp": wcp_h,
            "ws2": ws2_h,
            "bias": bias,
            "rt": np.ascontiguousarray(routing[sl].T),
            "rf": np.ascontiguousarray(routing[sl].reshape(1, BPC * E)),
        })
    return in_maps


def _run(in_maps, **kwargs):
    from concourse import bass_utils
    if "nc" not in _CACHE:
        _CACHE["nc"] = _build()
    return bass_utils.run_bass_kernel_spmd(
        _CACHE["nc"], in_maps, core_ids=list(range(NCORES)), **kwargs)


def kernel(x, routing_weights, weight, bias):
    in_maps = _prep_inputs(x, routing_weights, weight, bias)
    res = _run(in_maps)
    out = np.stack([res.results[c]["out"] for c in range(NCORES)], axis=0)
    return out.astype(np.float32).reshape(B, O, H, W)


# revision 5
# speedup vs baseline: 1.1973x; 1.1417x over previous
"""CondConv2d (MoE-routed 3x3 conv) Trainium2 Bass kernel.

Full-input contract: kernel(**inputs) takes the unsharded tensors and
returns the full [32, 192, 56, 56] output. Internally: data-parallel
across batch over 8 NeuronCores (4 samples per core). Each core mixes
its own per-sample weights on-device (routing @ expert_weights via
chained multiply-accumulate split across DVE and Pool; experts
replicated in SBUF) and runs its samples' convolutions as shifted bf16
matmuls accumulating in fp32 PSUM; PSUM is drained by the scalar
engine with the routed bias fused into the copy.

Conv decomposition per sample (I=O=192, K=3, H=W=56, pad=1):
  out[o, p] = sum over (i, dy, dx) of w[o, i, dy, dx] * xpad[i, h+dy, w+dx]
As matmuls with contraction on the SBUF partition dim (<=128):
  - channels i in [0,128): 9 chunks (one per (dy,dx)), K=128
  - channels i in [128,192) stored as shifted 2-copy tiles so pairs of
    offsets share one K=128 matmul:
      xt = [normal | row-shifted]  -> (dy=0,dy=1) pairs per dx: 3 chunks
      xc = [normal | col-shifted]  -> (dy=2,dx=0)|(dy=2,dx=1): 1 chunk
      leftover (dy=2,dx=2): 1 chunk, K=64 (reads xt[0:64])
  => 14 accumulating matmuls per (O-chunk, pixel-tile); O split 128+64.
Pixels tiled 7 x 448 (8 rows of 56), each tile in its own PSUM bank.

x arrives as flat [ch, 56*56] DMAs (large contiguous descriptors, no
small-element penalty) and is pad-copied into bordered [58, 58] tiles
by the scalar/pool engines. Sample 0's mixing uses a two-engine tree
(DVE experts 0-3, Pool experts 4-7, DVE combine) to halve the latency
the PE waits on; later samples use whole chains spread across both
engines. Sample 0's first O-chunk iterates chunk-outer so matmuls
start as soon as group-0 weights are mixed; every other O-chunk
iterates tile-outer so each PSUM bank's drain starts 2.6us after the
bank's first matmul, which keeps the PE free of PSUM write-after-read
stalls (and the p-state ramp resets those stalls would cause).
"""

import numpy as np

B, E = 32, 8
O, I = 192, 192
H, W = 56, 56
HP = H + 2  # padded side
NCORES = 8
BPC = B // NCORES  # samples per core
NT = 7  # pixel tiles per sample
RPT = 8  # output rows per pixel tile
TW = RPT * W  # 448 pixels per tile
NCH = 14  # accumulating matmul chunks per (O-chunk, pixel-tile)
OC = ((0, 128), (128, 64))  # (o_start, o_size) chunks

_CACHE = {}


def _build():
    import concourse.bass as bass  # noqa: F401
    from concourse import bacc, mybir, tile

    dt = mybir.dt
    f32 = dt.float32
    bf16 = dt.bfloat16
    MULT = mybir.AluOpType.mult
    ADD = mybir.AluOpType.add
    IDENT = mybir.ActivationFunctionType.Identity

    nc = bacc.Bacc(
        "TRN2",
        target_bir_lowering=False,
        debug=False,
        enable_asserts=False,
        num_devices=NCORES,
    )

    xin = nc.dram_tensor("xin", [BPC, I, H, W], bf16, kind="ExternalInput").ap()
    # wht free layout: ((g*E + e)*3 + d)*O + o with off = g*3 + d
    wht_d = nc.dram_tensor("wht", [128, 9 * E * O], bf16, kind="ExternalInput").ap()
    # wtp free layout: e*(3*O) + dx*O + o; p<64: (ch 128+p, dy0); p>=64: dy1
    wtp_d = nc.dram_tensor("wtp", [128, 3 * E * O], bf16, kind="ExternalInput").ap()
    # wcp free layout: e*O + o; p<64: (ch 128+p, dy2, dx0); p>=64: (dy2, dx1)
    wcp_d = nc.dram_tensor("wcp", [128, E * O], bf16, kind="ExternalInput").ap()
    # ws2 free layout: e*O + o; (ch 128+p, dy2, dx2)
    ws2_d = nc.dram_tensor("ws2", [64, E * O], bf16, kind="ExternalInput").ap()
    bias_d = nc.dram_tensor("bias", [E, O], f32, kind="ExternalInput").ap()
    rt_d = nc.dram_tensor("rt", [E, BPC], f32, kind="ExternalInput").ap()
    rf_d = nc.dram_tensor("rf", [1, BPC * E], f32, kind="ExternalInput").ap()
    out_d = nc.dram_tensor("out", [BPC, O, H * W], bf16, kind="ExternalOutput").ap()

    with tile.TileContext(nc) as tc:
        with (
            tc.tile_pool(name="consts", bufs=1) as consts,
            tc.tile_pool(name="wm", bufs=3) as wm_pool,
            tc.tile_pool(name="stage", bufs=2) as stage_pool,
            tc.tile_pool(name="psum1", bufs=1, space="PSUM") as psum1,
            tc.tile_pool(name="cpsum", bufs=1, space="PSUM") as cpsum,
        ):
            # ---- small inputs (needed for broadcast/bias matmuls)
            bias_sb = consts.tile([E, O], f32)
            nc.sync.dma_start(out=bias_sb, in_=bias_d)
            rt_sb = consts.tile([E, BPC], f32)
            nc.sync.dma_start(out=rt_sb, in_=rt_d)
            rf_sb = consts.tile([1, BPC * E], f32)
            nc.sync.dma_start(out=rf_sb, in_=rf_d)

            # ---- broadcast routing to all partitions + mix bias, via matmuls
            ones_sb = consts.tile([1, 128], f32)
            nc.vector.memset(ones_sb, 1.0)
            NRB = BPC * E
            ps0 = psum1.tile([128, 2 * BPC + NRB], f32)  # one PSUM bank
            nc.tensor.matmul(ps0[:, 0:BPC], lhsT=bias_sb[:, 0:128], rhs=rt_sb,
                             start=True, stop=True)
            nc.tensor.matmul(ps0[0:64, BPC:2 * BPC], lhsT=bias_sb[:, 128:192],
                             rhs=rt_sb, start=True, stop=True)
            nc.tensor.matmul(ps0[:, 2 * BPC:], lhsT=ones_sb, rhs=rf_sb,
                             start=True, stop=True)
            bias_cols = consts.tile([128, 2 * BPC], f32)
            nc.vector.tensor_copy(bias_cols[:, 0:BPC], ps0[:, 0:BPC])
            nc.vector.tensor_copy(bias_cols[0:64, BPC:2 * BPC], ps0[0:64, BPC:2 * BPC])
            rb = consts.tile([128, NRB], f32)
            nc.vector.tensor_copy(rb, ps0[:, 2 * BPC:])

            # ---- resident expert weights + per-sample x tiles.
            wht = consts.tile([128, 3, 3 * E * O], bf16)  # [i, g, (e, d, o)]
            wtp = consts.tile([128, E, 3 * O], bf16)  # [i2pair, e, (dx, o)]
            wcp = consts.tile([128, E, O], bf16)  # [i2cpair, e, o]
            ws2 = consts.tile([64, E, O], bf16)

            def emit_wht_dma(g):
                for e in range(E):
                    base = (g * E + e) * (3 * O)
                    nc.sync.dma_start(out=wht[:, g, e * 3 * O:(e + 1) * 3 * O],
                                      in_=wht_d[:, base:base + 3 * O])

            # flat x staging (contiguous DMA, no small-descriptor penalty)
            fh = [consts.tile([128, H, W], bf16, tag=f"fh{i}", name=f"fh{i}")
                  for i in range(2)]
            ft = [consts.tile([64, H, W], bf16, tag=f"ft{i}", name=f"ft{i}")
                  for i in range(2)]
            xh = [consts.tile([128, HP, HP], bf16, tag=f"xh{i}", name=f"xh{i}")
                  for i in range(2)]
            xt = [consts.tile([128, HP, HP], bf16, tag=f"xt{i}", name=f"xt{i}")
                  for i in range(2)]
            xc = [consts.tile([128, HP, HP], bf16, tag=f"xc{i}", name=f"xc{i}")
                  for i in range(2)]
            for t_ in xh + xt + xc:
                nc.gpsimd.memset(t_[:, 0, :], 0.0)
                nc.gpsimd.memset(t_[:, HP - 1, :], 0.0)
                nc.gpsimd.memset(t_[:, :, 0], 0.0)
                nc.gpsimd.memset(t_[:, :, HP - 1], 0.0)

            def emit_xflat_dma(b):
                nc.sync.dma_start(out=fh[b % 2], in_=xin[b, 0:128])
                nc.sync.dma_start(out=ft[b % 2], in_=xin[b, 128:192])

            def emit_pads(b, critical):
                i = b % 2
                fhb, ftb = fh[i], ft[i]
                xhb, xtb, xcb = xh[i], xt[i], xc[i]
                nc.scalar.activation(xhb[:, 1:H + 1, 1:W + 1], fhb, IDENT)
                nc.scalar.activation(xtb[0:64, 1:H + 1, 1:W + 1], ftb, IDENT)
                if critical:  # keep Pool free for mixing halves at startup
                    nc.scalar.activation(xtb[64:128, 0:H, 1:W + 1], ftb, IDENT)
                    nc.scalar.activation(xcb[0:64, 1:H + 1, 1:W + 1], ftb, IDENT)
                    nc.scalar.activation(xcb[64:128, 1:H + 1, 0:W], ftb, IDENT)
                else:
                    nc.gpsimd.tensor_copy(xtb[64:128, 0:H, 1:W + 1], ftb)
                    nc.scalar.activation(xcb[0:64, 1:H + 1, 1:W + 1], ftb, IDENT)
                    nc.gpsimd.tensor_copy(xcb[64:128, 1:H + 1, 0:W], ftb)

            emit_wht_dma(0)
            emit_xflat_dma(0)
            emit_wht_dma(1)
            emit_wht_dma(2)
            for e in range(E):
                nc.sync.dma_start(out=wtp[:, e, :],
                                  in_=wtp_d[:, e * 3 * O:(e + 1) * 3 * O])
            nc.sync.dma_start(out=wcp, in_=wcp_d)
            nc.sync.dma_start(out=ws2, in_=ws2_d)

            # ---- mixing: routed per-sample weights
            GROUPS = (
                ("wmg0", 3 * O, 128, lambda e: wht[:, 0, e * 3 * O:(e + 1) * 3 * O]),
                ("wmg1", 3 * O, 128, lambda e: wht[:, 1, e * 3 * O:(e + 1) * 3 * O]),
                ("wmg2", 3 * O, 128, lambda e: wht[:, 2, e * 3 * O:(e + 1) * 3 * O]),
                ("wmp", 3 * O, 128, lambda e: wtp[:, e, :]),
                ("wmc", O, 128, lambda e: wcp[:, e, :]),
                ("wms", O, 64, lambda e: ws2[:, e, :]),
            )
            def _chain(eng, dst, src, b, es):
                for k, e in enumerate(es):
                    rc = rb[0:dst.shape[0], b * E + e:b * E + e + 1]
                    if k == 0:
                        eng.tensor_scalar_mul(dst, src(e), rc)
                    else:
                        eng.scalar_tensor_tensor(dst, src(e), rc, dst,
                                                 op0=MULT, op1=ADD)

            def emit_mix(b):
                # per-partition-scalar ops only exist on DVE; chains emitted
                # in the conv's consumption order
                out = []
                for tag, width, np_, src in GROUPS:
                    dst = wm_pool.tile([np_, width], bf16, tag=tag, name=tag)
                    _chain(nc.vector, dst, src, b, range(8))
                    out.append(dst)
                return out

            def conv_ochunk(b, oci, tile_outer):
                xhb, xtb, xcb = xh[b % 2], xt[b % 2], xc[b % 2]
                wmg0, wmg1, wmg2, wmp, wmc, wms = wm[b]
                wmg = (wmg0, wmg1, wmg2)
                o0, osz = OC[oci]
                pst = [cpsum.tile([128, 512], f32, tag=f"cps{t}",
                                  name=f"cps{t}") for t in range(NT)]
                bc = (bias_cols[:, b:b + 1] if osz == 128
                      else bias_cols[0:64, BPC + b:BPC + b + 1])
                st = stage_pool.tile([128, H * W], bf16, tag="st", name="st")

                def emit_chunk(ci, t):
                    kw = dict(start=(ci == 0), stop=(ci == NCH - 1))
                    po = pst[t][0:osz, 0:TW]
                    if ci < 9:
                        dy, dx = divmod(ci, 3)
                        nc.tensor.matmul(
                            po, lhsT=wmg[dy][:, dx * O + o0:dx * O + o0 + osz],
                            rhs=xhb[:, t * RPT + dy:t * RPT + dy + RPT,
                                    dx:dx + W], **kw)
                    elif ci < 12:
                        dx = ci - 9
                        nc.tensor.matmul(
                            po, lhsT=wmp[:, dx * O + o0:dx * O + o0 + osz],
                            rhs=xtb[:, t * RPT:t * RPT + RPT, dx:dx + W], **kw)
                    elif ci == 12:
                        nc.tensor.matmul(
                            po, lhsT=wmc[:, o0:o0 + osz],
                            rhs=xcb[:, t * RPT + 2:t * RPT + 2 + RPT, 0:W], **kw)
                    else:
                        nc.tensor.matmul(
                            po, lhsT=wms[0:64, o0:o0 + osz],
                            rhs=xtb[0:64, t * RPT + 2:t * RPT + 2 + RPT,
                                    2:2 + W], **kw)

                def emit_drain(t):
                    nc.scalar.activation(
                        st[0:osz, t * TW:(t + 1) * TW],
                        pst[t][0:osz, 0:TW], IDENT, bias=bc)

                if tile_outer:
                    # per-tile chunk runs let each bank's drain start 2.6us
                    # into the set -> no PSUM WAR stalls at set boundaries
                    for t in range(NT):
                        for ci in range(NCH):
                            emit_chunk(ci, t)
                        emit_drain(t)
                else:
                    # chunk-outer: consumes mixed groups/x in arrival order
                    for ci in range(NCH):
                        for t in range(NT):
                            emit_chunk(ci, t)
                    for t in range(NT):
                        emit_drain(t)
                nc.sync.dma_start(out=out_d[b, o0:o0 + osz, :],
                                  in_=st[0:osz, :])

            emit_pads(0, critical=True)
            wm = {0: emit_mix(0)}

            for b in range(BPC):
                if b + 1 < BPC:
                    emit_xflat_dma(b + 1)
                conv_ochunk(b, 0, tile_outer=(b > 0))
                if b + 1 < BPC:
                    emit_pads(b + 1, critical=False)
                    wm[b + 1] = emit_mix(b + 1)
                conv_ochunk(b, 1, tile_outer=True)
                del wm[b]

    nc.compile()
    return nc


def _prep_inputs(x, routing_weights, weight, bias):
    import ml_dtypes
    bf16 = ml_dtypes.bfloat16

    x = np.asarray(x, np.float32).astype(bf16)
    routing = np.asarray(routing_weights, np.float32)
    weight = np.asarray(weight, np.float32).astype(bf16)
    bias = np.asarray(bias, np.float32)

    W5 = weight.reshape(E, O, I, 3, 3)
    # head: [i, dy(g), e, dx(d), o] -> f = ((g*E + e)*3 + d)*O + o
    wht_h = np.ascontiguousarray(
        W5[:, :, :128].transpose(2, 3, 0, 4, 1)).reshape(128, 9 * E * O)
    # tail row pair: p<64 -> (i=128+p, dy=0); p>=64 -> (i=64+p, dy=1)
    # f = e*(3*O) + dx*O + o
    t0 = W5[:, :, 128:, 0, :].transpose(2, 0, 3, 1)  # [i2, e, dx, o]
    t1 = W5[:, :, 128:, 1, :].transpose(2, 0, 3, 1)
    wtp_h = np.ascontiguousarray(
        np.concatenate([t0, t1], axis=0)).reshape(128, 3 * E * O)
    # tail col pair: p<64 -> (dy=2, dx=0); p>=64 -> (dy=2, dx=1); f = e*O + o
    c0 = W5[:, :, 128:, 2, 0].transpose(2, 0, 1)  # [i2, e, o]
    c1 = W5[:, :, 128:, 2, 1].transpose(2, 0, 1)
    wcp_h = np.ascontiguousarray(
        np.concatenate([c0, c1], axis=0)).reshape(128, E * O)
    ws2_h = np.ascontiguousarray(
        W5[:, :, 128:, 2, 2].transpose(2, 0, 1)).reshape(64, E * O)

    in_maps = []
    for c in range(NCORES):
        sl = slice(c * BPC, (c + 1) * BPC)
        in_maps.append({
            "xin": np.ascontiguousarray(x[sl]),
            "wht": wht_h,
            "wtp": wtp_h,
            "wcp": wcp_h,
            "ws2": ws2_h,
            "bias": bias,
            "rt": np.ascontiguousarray(routing[sl].T),
            "rf": np.ascontiguousarray(routing[sl].reshape(1, BPC * E)),
        })
    return in_maps


def _run(in_maps, **kwargs):
    from concourse import bass_utils
    if "nc" not in _CACHE:
        _CACHE["nc"] = _build()
    return bass_utils.run_bass_kernel_spmd(
        _CACHE["nc"], in_maps, core_ids=list(range(NCORES)), **kwargs)


def kernel(x, routing_weights, weight, bias):
    in_maps = _prep_inputs(x, routing_weights, weight, bias)
    res = _run(in_maps)
    out = np.stack([res.results[c]["out"] for c in range(NCORES)], axis=0)
    return out.astype(np.float32).reshape(B, O, H, W)


# revision 10
# speedup vs baseline: 1.2401x; 1.0357x over previous
"""CondConv2d (MoE-routed 3x3 conv) Trainium2 Bass kernel.

Full-input contract: kernel(**inputs) takes the unsharded tensors and
returns the full [32, 192, 56, 56] output. Internally: data-parallel
across batch over 8 NeuronCores (4 samples per core). Each core mixes
its own per-sample weights on-device (routing @ expert_weights via
chained multiply-accumulate split across DVE and Pool; experts
replicated in SBUF) and runs its samples' convolutions as shifted bf16
matmuls accumulating in fp32 PSUM; PSUM is drained by the scalar
engine with the routed bias fused into the copy.

Conv decomposition per sample (I=O=192, K=3, H=W=56, pad=1):
  out[o, p] = sum over (i, dy, dx) of w[o, i, dy, dx] * xpad[i, h+dy, w+dx]
As matmuls with contraction on the SBUF partition dim (<=128):
  - channels i in [0,128): 9 chunks (one per (dy,dx)), K=128
  - channels i in [128,192) stored as shifted 2-copy tiles so pairs of
    offsets share one K=128 matmul:
      xt = [normal | row-shifted]  -> (dy=0,dy=1) pairs per dx: 3 chunks
      xc = [normal | col-shifted]  -> (dy=2,dx=0)|(dy=2,dx=1): 1 chunk
      leftover (dy=2,dx=2): 1 chunk, K=64 (reads xt[0:64])
  => 14 accumulating matmuls per (O-chunk, pixel-tile); O split 128+64.
Pixels tiled 7 x 448 (8 rows of 56), each tile in its own PSUM bank.

x arrives as flat [ch, 56*56] DMAs (large contiguous descriptors, no
small-element penalty) and is pad-copied into bordered [58, 58] tiles
by the scalar/pool engines. Sample 0's mixing uses a two-engine tree
(DVE experts 0-3, Pool experts 4-7, DVE combine) to halve the latency
the PE waits on; later samples use whole chains spread across both
engines. Sample 0's first O-chunk iterates chunk-outer so matmuls
start as soon as group-0 weights are mixed; every other O-chunk
iterates tile-outer so each PSUM bank's drain starts 2.6us after the
bank's first matmul, which keeps the PE free of PSUM write-after-read
stalls (and the p-state ramp resets those stalls would cause).
"""

import numpy as np

B, E = 32, 8
O, I = 192, 192
H, W = 56, 56
HP = H + 2  # padded side
NCORES = 8
BPC = B // NCORES  # samples per core
NT = 7  # pixel tiles per sample
RPT = 8  # output rows per pixel tile
TW = RPT * W  # 448 pixels per tile
NCH = 14  # accumulating matmul chunks per (O-chunk, pixel-tile)
OC = ((0, 128), (128, 64))  # (o_start, o_size) chunks

_CACHE = {}


def _build():
    import concourse.bass as bass  # noqa: F401
    from concourse import bacc, mybir, tile

    dt = mybir.dt
    f32 = dt.float32
    bf16 = dt.bfloat16
    MULT = mybir.AluOpType.mult
    ADD = mybir.AluOpType.add
    IDENT = mybir.ActivationFunctionType.Identity

    nc = bacc.Bacc(
        "TRN2",
        target_bir_lowering=False,
        debug=False,
        enable_asserts=False,
        num_devices=NCORES,
    )

    xin = nc.dram_tensor("xin", [BPC, I, H, W], bf16, kind="ExternalInput").ap()
    # wht free layout: ((g*E + e)*3 + d)*O + o with off = g*3 + d
    wht_d = nc.dram_tensor("wht", [128, 9 * E * O], bf16, kind="ExternalInput").ap()
    # wtp free layout: e*(3*O) + dx*O + o; p<64: (ch 128+p, dy0); p>=64: dy1
    wtp_d = nc.dram_tensor("wtp", [128, 3 * E * O], bf16, kind="ExternalInput").ap()
    # wcp free layout: e*O + o; p<64: (ch 128+p, dy2, dx0); p>=64: (dy2, dx1)
    wcp_d = nc.dram_tensor("wcp", [128, E * O], bf16, kind="ExternalInput").ap()
    # ws2 free layout: e*O + o; (ch 128+p, dy2, dx2)
    ws2_d = nc.dram_tensor("ws2", [64, E * O], bf16, kind="ExternalInput").ap()
    bias_d = nc.dram_tensor("bias", [E, O], f32, kind="ExternalInput").ap()
    rt_d = nc.dram_tensor("rt", [E, BPC], f32, kind="ExternalInput").ap()
    rf_d = nc.dram_tensor("rf", [1, BPC * E], f32, kind="ExternalInput").ap()
    out_d = nc.dram_tensor("out", [BPC, O, H * W], bf16, kind="ExternalOutput").ap()

    with tile.TileContext(nc) as tc:
        with (
            tc.tile_pool(name="consts", bufs=1) as consts,
            tc.tile_pool(name="wm", bufs=3) as wm_pool,
            tc.tile_pool(name="stage", bufs=2) as stage_pool,
            tc.tile_pool(name="psum1", bufs=1, space="PSUM") as psum1,
            tc.tile_pool(name="cpsum", bufs=1, space="PSUM") as cpsum,
        ):
            # ---- small inputs (needed for broadcast/bias matmuls)
            bias_sb = consts.tile([E, O], f32)
            nc.sync.dma_start(out=bias_sb, in_=bias_d)
            rt_sb = consts.tile([E, BPC], f32)
            nc.sync.dma_start(out=rt_sb, in_=rt_d)
            rf_sb = consts.tile([1, BPC * E], f32)
            nc.sync.dma_start(out=rf_sb, in_=rf_d)

            # ---- broadcast routing to all partitions + mix bias, via matmuls
            ones_sb = consts.tile([1, 128], f32)
            nc.vector.memset(ones_sb, 1.0)
            NRB = BPC * E
            # startup scratch shares the bank later used by sample-0's
            # O-tail t0/t1 pair, keeping conv PSUM within 8 banks
            ps0 = cpsum.tile([128, 512], f32, tag="cpsOT", name="ps0")
            nc.tensor.matmul(ps0[:, 0:BPC], lhsT=bias_sb[:, 0:128], rhs=rt_sb,
                             start=True, stop=True)
            nc.tensor.matmul(ps0[0:64, BPC:2 * BPC], lhsT=bias_sb[:, 128:192],
                             rhs=rt_sb, start=True, stop=True)
            nc.tensor.matmul(ps0[64:128, BPC:2 * BPC], lhsT=bias_sb[:, 128:192],
                             rhs=rt_sb, start=True, stop=True)
            nc.tensor.matmul(ps0[:, 2 * BPC:2 * BPC + NRB], lhsT=ones_sb,
                             rhs=rf_sb, start=True, stop=True)
            bias_cols = consts.tile([128, 2 * BPC], f32)
            nc.vector.tensor_copy(bias_cols, ps0[:, 0:2 * BPC])
            rb = consts.tile([128, NRB], f32)
            nc.vector.tensor_copy(rb, ps0[:, 2 * BPC:2 * BPC + NRB])

            # ---- resident expert weights + per-sample x tiles.
            wht = consts.tile([128, 3, 3 * E * O], bf16)  # [i, g, (e, d, o)]
            wtp = consts.tile([128, E, 3 * O], bf16)  # [i2pair, e, (dx, o)]
            wcp = consts.tile([128, E, O], bf16)  # [i2cpair, e, o]
            ws2 = consts.tile([64, E, O], bf16)

            def emit_wht_dma(g):
                # two expert-half batches: HWDGE fixed cost (~0.63us per
                # dma_start) dominates per-expert chunks
                half = 4 * 3 * O
                base = g * E * 3 * O
                nc.sync.dma_start(out=wht[:, g, 0:half],
                                  in_=wht_d[:, base:base + half])
                nc.sync.dma_start(out=wht[:, g, half:2 * half],
                                  in_=wht_d[:, base + half:base + 2 * half])

            # flat x staging (contiguous DMA, no small-descriptor penalty)
            fh = [consts.tile([128, H, W], bf16, tag=f"fh{i}", name=f"fh{i}")
                  for i in range(2)]
            ft = [consts.tile([64, H, W], bf16, tag=f"ft{i}", name=f"ft{i}")
                  for i in range(2)]
            xh = [consts.tile([128, HP, HP], bf16, tag=f"xh{i}", name=f"xh{i}")
                  for i in range(2)]
            xt = [consts.tile([128, HP, HP], bf16, tag=f"xt{i}", name=f"xt{i}")
                  for i in range(2)]
            xc = [consts.tile([128, HP, HP], bf16, tag=f"xc{i}", name=f"xc{i}")
                  for i in range(2)]
            for t_ in xh + xt + xc:
                nc.gpsimd.memset(t_[:, 0, :], 0.0)
                nc.gpsimd.memset(t_[:, HP - 1, :], 0.0)
                nc.gpsimd.memset(t_[:, :, 0], 0.0)
                nc.gpsimd.memset(t_[:, :, HP - 1], 0.0)

            def emit_xflat_dma(b):
                nc.sync.dma_start(out=fh[b % 2], in_=xin[b, 0:128])
                nc.sync.dma_start(out=ft[b % 2], in_=xin[b, 128:192])

            def emit_pads(b, critical):
                i = b % 2
                fhb, ftb = fh[i], ft[i]
                xhb, xtb, xcb = xh[i], xt[i], xc[i]
                nc.scalar.activation(xhb[:, 1:H + 1, 1:W + 1], fhb, IDENT)
                nc.scalar.activation(xtb[0:64, 1:H + 1, 1:W + 1], ftb, IDENT)
                if critical:  # keep Pool free for mixing halves at startup
                    nc.scalar.activation(xtb[64:128, 0:H, 1:W + 1], ftb, IDENT)
                    nc.scalar.activation(xcb[0:64, 1:H + 1, 1:W + 1], ftb, IDENT)
                    nc.scalar.activation(xcb[64:128, 1:H + 1, 0:W], ftb, IDENT)
                else:
                    nc.gpsimd.tensor_copy(xtb[64:128, 0:H, 1:W + 1], ftb)
                    nc.scalar.activation(xcb[0:64, 1:H + 1, 1:W + 1], ftb, IDENT)
                    nc.gpsimd.tensor_copy(xcb[64:128, 1:H + 1, 0:W], ftb)

            emit_wht_dma(0)
            nc.sync.dma_start(out=fh[0], in_=xin[0, 0:128])
            emit_wht_dma(1)
            nc.sync.dma_start(out=ft[0], in_=xin[0, 128:192])
            emit_wht_dma(2)
            nc.sync.dma_start(out=wtp[:, 0:4, :], in_=wtp_d[:, 0:4 * 3 * O])
            nc.sync.dma_start(out=wtp[:, 4:8, :], in_=wtp_d[:, 4 * 3 * O:])
            nc.sync.dma_start(out=wcp, in_=wcp_d)
            nc.sync.dma_start(out=ws2, in_=ws2_d)

            # ---- mixing: routed per-sample weights
            GROUPS = (
                ("wmg0", 3 * O, 128, lambda e: wht[:, 0, e * 3 * O:(e + 1) * 3 * O]),
                ("wmg1", 3 * O, 128, lambda e: wht[:, 1, e * 3 * O:(e + 1) * 3 * O]),
                ("wmg2", 3 * O, 128, lambda e: wht[:, 2, e * 3 * O:(e + 1) * 3 * O]),
                ("wmp", 3 * O, 128, lambda e: wtp[:, e, :]),
                ("wmc", O, 128, lambda e: wcp[:, e, :]),
                ("wms", O, 64, lambda e: ws2[:, e, :]),
            )
            def _chain(eng, dst, src, b, es):
                for k, e in enumerate(es):
                    rc = rb[0:dst.shape[0], b * E + e:b * E + e + 1]
                    if k == 0:
                        eng.tensor_scalar_mul(dst, src(e), rc)
                    else:
                        eng.scalar_tensor_tensor(dst, src(e), rc, dst,
                                                 op0=MULT, op1=ADD)

            def emit_mix(b):
                # per-partition-scalar ops only exist on DVE; chains emitted
                # in the conv's consumption order
                out = []
                for tag, width, np_, src in GROUPS:
                    dst = wm_pool.tile([np_, width], bf16, tag=tag, name=tag)
                    _chain(nc.vector, dst, src, b, range(8))
                    out.append(dst)
                return out

            def emit_chunk(b, ci, t, po, o0, osz):
                xhb, xtb, xcb = xh[b % 2], xt[b % 2], xc[b % 2]
                wmg0, wmg1, wmg2, wmp, wmc, wms = wm[b]
                wmg = (wmg0, wmg1, wmg2)
                kw = dict(start=(ci == 0), stop=(ci == NCH - 1))
                if ci < 9:
                    dy, dx = divmod(ci, 3)
                    nc.tensor.matmul(
                        po, lhsT=wmg[dy][:, dx * O + o0:dx * O + o0 + osz],
                        rhs=xhb[:, t * RPT + dy:t * RPT + dy + RPT,
                                dx:dx + W], **kw)
                elif ci < 12:
                    dx = ci - 9
                    nc.tensor.matmul(
                        po, lhsT=wmp[:, dx * O + o0:dx * O + o0 + osz],
                        rhs=xtb[:, t * RPT:t * RPT + RPT, dx:dx + W], **kw)
                elif ci == 12:
                    nc.tensor.matmul(
                        po, lhsT=wmc[:, o0:o0 + osz],
                        rhs=xcb[:, t * RPT + 2:t * RPT + 2 + RPT, 0:W], **kw)
                else:
                    nc.tensor.matmul(
                        po, lhsT=wms[0:64, o0:o0 + osz],
                        rhs=xtb[0:64, t * RPT + 2:t * RPT + 2 + RPT,
                                2:2 + W], **kw)

            def conv_ochunk(b, oci, tile_outer):
                o0, osz = OC[oci]
                pst = [cpsum.tile([128, 512], f32, tag=f"cps{t}",
                                  name=f"cps{t}") for t in range(NT)]
                bc = (bias_cols[:, b:b + 1] if osz == 128
                      else bias_cols[0:64, BPC + b:BPC + b + 1])
                tag = "sth" if oci == 0 else "stt"
                st = stage_pool.tile([128, H * W], bf16, tag=tag, name=tag)

                def emit_drain(t):
                    nc.scalar.activation(
                        st[0:osz, t * TW:(t + 1) * TW],
                        pst[t][0:osz, 0:TW], IDENT, bias=bc)

                if tile_outer:
                    # per-tile chunk runs let each bank's drain start 2.6us
                    # into the set -> no PSUM WAR stalls at set boundaries
                    for t in range(NT):
                        for ci in range(NCH):
                            emit_chunk(b, ci, t, pst[t][0:osz, 0:TW], o0, osz)
                        emit_drain(t)
                else:
                    # chunk-outer: consumes mixed groups/x in arrival order
                    for ci in range(NCH):
                        for t in range(NT):
                            emit_chunk(b, ci, t, pst[t][0:osz, 0:TW], o0, osz)
                    for t in range(NT):
                        emit_drain(t)
                nc.sync.dma_start(out=out_d[b, o0:o0 + osz, :],
                                  in_=st[0:osz, :])

            # sample 0 runs against mixing still in flight: interleave the
            # O-head with two O-tail tiles so each mixed group is consumed
            # over ~5us (matching the DVE chain rate), then finish the
            # remaining O-tail tiles once everything is resident.
            s0_state = {}

            def conv_s0_part1():
                psth = [cpsum.tile([128, 512], f32, tag=f"cps{t}",
                                   name=f"cps{t}") for t in range(NT)]
                pstot = cpsum.tile([128, 512], f32, tag="cpsOT", name="cpsOT")
                for ci in range(NCH):
                    for t in range(NT):
                        emit_chunk(0, ci, t, psth[t][0:128, 0:TW], 0, 128)
                    emit_chunk(0, ci, 0, pstot[0:64, 0:TW], 128, 64)
                    emit_chunk(0, ci, 1, pstot[64:128, 0:TW], 128, 64)
                sth = stage_pool.tile([128, H * W], bf16, tag="sth", name="sth")
                for t in range(NT):
                    nc.scalar.activation(
                        sth[:, t * TW:(t + 1) * TW], psth[t][:, 0:TW],
                        IDENT, bias=bias_cols[:, 0:1])
                nc.sync.dma_start(out=out_d[0, 0:128, :], in_=sth)
                stt = stage_pool.tile([128, H * W], bf16, tag="stt", name="stt")
                nc.scalar.activation(stt[0:64, 0:TW], pstot[0:64, 0:TW],
                                     IDENT, bias=bias_cols[0:64, BPC:BPC + 1])
                nc.scalar.activation(stt[64:128, TW:2 * TW],
                                     pstot[64:128, 0:TW], IDENT,
                                     bias=bias_cols[64:128, BPC:BPC + 1])
                s0_state["stt"] = stt

            def conv_s0_part2():
                stt = s0_state["stt"]
                for t in range(2, NT):
                    pst = cpsum.tile([128, 512], f32, tag=f"cps{t - 2}",
                                     name=f"cps{t - 2}")
                    for ci in range(NCH):
                        emit_chunk(0, ci, t, pst[0:64, 0:TW], 128, 64)
                    nc.scalar.activation(
                        stt[0:64, t * TW:(t + 1) * TW], pst[0:64, 0:TW],
                        IDENT, bias=bias_cols[0:64, BPC:BPC + 1])
                nc.sync.dma_start(out=out_d[0, 128:192, 0:TW],
                                  in_=stt[0:64, 0:TW])
                nc.sync.dma_start(out=out_d[0, 128:192, TW:2 * TW],
                                  in_=stt[64:128, TW:2 * TW])
                nc.sync.dma_start(out=out_d[0, 128:192, 2 * TW:],
                                  in_=stt[0:64, 2 * TW:])

            emit_pads(0, critical=True)
            wm = {0: emit_mix(0)}

            for b in range(BPC):
                if b + 1 < BPC:
                    emit_xflat_dma(b + 1)
                if b == 0:
                    conv_s0_part1()
                else:
                    # b=1 still outruns its mixing with tile-outer, so its
                    # O-head stays chunk-outer (progressive group use)
                    conv_ochunk(b, 0, tile_outer=(b >= 2))
                if b + 1 < BPC:
                    emit_pads(b + 1, critical=False)
                    wm[b + 1] = emit_mix(b + 1)
                if b == 0:
                    conv_s0_part2()
                else:
                    conv_ochunk(b, 1, tile_outer=True)
                del wm[b]

    nc.compile()
    return nc


def _prep_inputs(x, routing_weights, weight, bias):
    import ml_dtypes
    bf16 = ml_dtypes.bfloat16

    x = np.asarray(x, np.float32).astype(bf16)
    routing = np.asarray(routing_weights, np.float32)
    weight = np.asarray(weight, np.float32).astype(bf16)
    bias = np.asarray(bias, np.float32)

    W5 = weight.reshape(E, O, I, 3, 3)
    # head: [i, dy(g), e, dx(d), o] -> f = ((g*E + e)*3 + d)*O + o
    wht_h = np.ascontiguousarray(
        W5[:, :, :128].transpose(2, 3, 0, 4, 1)).reshape(128, 9 * E * O)
    # tail row pair: p<64 -> (i=128+p, dy=0); p>=64 -> (i=64+p, dy=1)
    # f = e*(3*O) + dx*O + o
    t0 = W5[:, :, 128:, 0, :].transpose(2, 0, 3, 1)  # [i2, e, dx, o]
    t1 = W5[:, :, 128:, 1, :].transpose(2, 0, 3, 1)
    wtp_h = np.ascontiguousarray(
        np.concatenate([t0, t1], axis=0)).reshape(128, 3 * E * O)
    # tail col pair: p<64 -> (dy=2, dx=0); p>=64 -> (dy=2, dx=1); f = e*O + o
    c0 = W5[:, :, 128:, 2, 0].transpose(2, 0, 1)  # [i2, e, o]
    c1 = W5[:, :, 128:, 2, 1].transpose(2, 0, 1)
    wcp_h = np.ascontiguousarray(
        np.concatenate([c0, c1], axis=0)).reshape(128, E * O)
    ws2_h = np.ascontiguousarray(
        W5[:, :, 128:, 2, 2].transpose(2, 0, 1)).reshape(64, E * O)

    in_maps = []
    for c in range(NCORES):
        sl = slice(c * BPC, (c + 1) * BPC)
        in_maps.append({
            "xin": np.ascontiguousarray(x[sl]),
            "wht": wht_h,
            "wtp": wtp_h,
            "wcp": wcp_h,
            "ws2": ws2_h,
            "bias": bias,
            "rt": np.ascontiguousarray(routing[sl].T),
            "rf": np.ascontiguousarray(routing[sl].reshape(1, BPC * E)),
        })
    return in_maps


def _run(in_maps, **kwargs):
    from concourse import bass_utils
    if "nc" not in _CACHE:
        _CACHE["nc"] = _build()
    return bass_utils.run_bass_kernel_spmd(
        _CACHE["nc"], in_maps, core_ids=list(range(NCORES)), **kwargs)


def kernel(x, routing_weights, weight, bias):
    in_maps = _prep_inputs(x, routing_weights, weight, bias)
    res = _run(in_maps)
    out = np.stack([res.results[c]["out"] for c in range(NCORES)], axis=0)
    return out.astype(np.float32).reshape(B, O, H, W)
